# revision 1
# baseline (speedup 1.0000x reference)
"""ARIMA(2,1,2) eps-residual kernel for Trainium2 (8 NeuronCores, SPMD).

Reference semantics (per batch row of y[B=8192, T=4096]):
    yd = diff(y); target_s = y[s+3] - (1+phi0) y[s+2] - phi1 y[s+1] - mu
    eps_s = target_s - theta0 eps_{s-1} - theta1 eps_{s-2},  s in [0, T-3)
    out = [eps, 0, 0]   (shape [B, T-1], fp32)

Kernel math: the MA-inverse 1/(1 + theta0 z + theta1 z^2) has |roots| <= 0.19,
so its impulse response h is numerically exact after 48 taps (~1e-34). The
whole computation collapses to a banded matrix A (band [-3, +46] around the
diagonal) applied per row: eps = A @ y + bias, where bias is constant except
in the first 48 columns. The band fits within 128, so each [128 rows x 512
out-cols] tile is 4-5 PE matmuls.

Device pipeline per core (1024 rows, pure batch data-parallelism):
  - natural DMA loads of y [128, 1024] column-chunks (HWDGE)
  - PE transposes 128x128 blocks -> PSUM; DVE/ACT evacuate to SBUF, rounding
    to float32r (PE's full-rate 4-byte matmul format, 13-bit mantissa)
  - per (row-tile, pair of 512-wide out-blocks): 9-10 fp32r matmuls
    (lhsT = transposed-y block, moving rhs = band matrix G_d) accumulating
    into a 2-bank PSUM tile; a single DVE/ACT op evacuates both blocks and
    folds the bias; out blocks are offset by -3 columns so the +3 forward
    taps never cross the tile budget
  - one natural DMA store per row-tile (ACT/HWDGE queue)

Accuracy: float32r quantizes operands to 13-bit mantissa -> max rel err
~2.6e-4 vs the fp32 reference (fp32 PSUM accumulation is exact).
"""
import sys

for _p in ("/opt/trn_rl_repo",):
    if _p not in sys.path:
        sys.path.append(_p)

import numpy as np

B_FULL, T = 8192, 4096
N_CORES = 8
B_SH = B_FULL // N_CORES      # 1024 rows per core
S = T - 3                     # 4093 valid eps columns
T_OUT = T - 1                 # 4095 output columns
P = 128
KH = 48                       # MA-inverse taps kept (exact to ~1e-34)
NROT = 17                     # live transposed-y tiles: two full chunks + prev


def host_constants(phi, theta, mu):
    """Band matrices G (9 classes of [128, 512]), first-block bias row, const bias."""
    phi = np.asarray(phi, np.float64)
    theta = np.asarray(theta, np.float64)
    mu = float(np.asarray(mu).reshape(-1)[0])
    h = np.zeros(KH)
    h[0] = 1.0
    for k in range(1, KH):
        h[k] = -theta[0] * h[k - 1] - (theta[1] * h[k - 2] if k >= 2 else 0.0)
    H = np.cumsum(h)
    c = {1: -phi[1], 2: -(1.0 + phi[0]), 3: 1.0}

    def astd(r):  # steady-state coefficient at lag r = s - j
        v = 0.0
        for m in (1, 2, 3):
            k = r + m
            if 0 <= k < KH:
                v += c[m] * h[k]
        return v

    p = np.arange(P)[:, None]
    q = np.arange(512)[None, :]
    gmats = np.zeros((9, P, 512), np.float32)
    # standard blocks (m>=1, out col s = 512m - 3 + q): d in {-1,0,1,2,3}
    rtab = np.array([astd(r) if -3 <= r <= KH - 2 else 0.0
                     for r in range(-1040, 1041)])
    for di, d in enumerate((-1, 0, 1, 2, 3)):
        r = q - 3 - 128 * d - p
        gmats[di] = rtab[r + 1040]
    # first block (m=0, s = q, truncated start, cols >= 509 zero): d in {0..3}
    for di, d in enumerate((0, 1, 2, 3)):
        s = q
        j = 128 * d + p
        G = np.zeros((P, 512))
        for m in (1, 2, 3):
            k = s + m - j
            ks = np.minimum(s, KH - 1)
            valid = (k >= 0) & (k <= ks)
            hk = np.where(valid, np.take(h, np.clip(k, 0, KH - 1)), 0.0)
            G = G + c[m] * hk
        G[np.broadcast_to(s >= 509, G.shape)] = 0.0
        gmats[5 + di] = G.astype(np.float32)

    bias0 = (-mu * H[np.minimum(np.arange(512), KH - 1)]).astype(np.float32)
    bias0[509:] = 0.0
    bias_const = float(-mu * H[KH - 1])
    # pre-round G to float32r's 12-bit-truncated mantissa (device copy re-rounds
    # identically, keeping host/device agreement)
    gv = gmats.view(np.uint32)
    gv &= np.uint32(0xFFFFF000)
    return gmats, np.broadcast_to(bias0.reshape(1, 512), (P, 512)).copy(), bias_const


def build_program(bias_const, reps=1, internal=False):
    import concourse.bacc as bacc
    import concourse.mybir as mybir
    from concourse.tile import TileContext
    from concourse import masks

    f32 = mybir.dt.float32
    f32r = mybir.dt.float32r
    alu = mybir.AluOpType

    nc = bacc.Bacc()
    g_in = nc.declare_dram_parameter("gmats", [9 * P, 512], f32, isOutput=False)
    b_in = nc.declare_dram_parameter("bias0", [P, 512], f32, isOutput=False)
    if internal:
        yio = nc.declare_dram_parameter("yio", [1, 4], f32, isOutput=True)
        y_in = nc.dram_tensor("ydr", [B_SH, T], f32)
        out = nc.dram_tensor("odr", [B_SH, T_OUT], f32)
    else:
        y_in = nc.declare_dram_parameter("y", [B_SH, T], f32, isOutput=False)
        out = nc.declare_dram_parameter("out", [B_SH, T_OUT], f32, isOutput=True)

    with TileContext(nc) as tc:
        with (
            tc.tile_pool(name="consts", bufs=1) as cpool,
            tc.tile_pool(name="ld", bufs=2) as ldpool,
            tc.tile_pool(name="yt", bufs=1) as ytpool,
            tc.tile_pool(name="ro", bufs=1) as ropool,
            tc.tile_pool(name="pst", bufs=3, space="PSUM") as pstg,
            tc.tile_pool(name="pacc", bufs=2, space="PSUM") as pacc,
        ):
            ident = cpool.tile([P, P], f32)
            masks.make_identity(nc, ident[:])
            bias0 = cpool.tile([P, 512], f32)
            nc.sync.dma_start(out=bias0[:], in_=b_in[:])
            gstage = cpool.tile([P, 9 * 512], f32)
            for gi_ in range(9):
                nc.sync.dma_start(
                    out=gstage[:, gi_ * 512:(gi_ + 1) * 512],
                    in_=g_in[gi_ * P:(gi_ + 1) * P, :])
            gr = cpool.tile([P, 9 * 512], f32r)
            nc.vector.tensor_copy(out=gr[:], in_=gstage[:])

            if internal:
                zf = cpool.tile([P, 1024], f32)
                nc.vector.memset(zf[:], 0.0)
                for i in range(B_SH // P):
                    for cc in range(4):
                        nc.sync.dma_start(
                            out=y_in[i * P:(i + 1) * P, cc * 1024:(cc + 1) * 1024],
                            in_=zf[:])

            def gmat(idx):
                return gr[:, idx * 512:(idx + 1) * 512]

            def body():
                yts = {}
                for grp in range(2):              # 2 groups of 4 row-tiles
                    rowouts = {}
                    for j in range(4):
                        ro_j = ropool.tile([P, T_OUT], f32, tag=f"ro{j}")
                        rowouts[j] = ro_j
                    ychunks = []
                    for tc_i in range(4):         # 1024-col t-chunks
                        if tc_i % 2 == 0:         # 2048-col (1 MiB) loads
                            ychunks = []
                            for j in range(4):
                                row0 = (grp * 4 + j) * P
                                yt = ldpool.tile([P, 2048], f32, tag=f"ych{j}")
                                nc.sync.dma_start(
                                    out=yt[:],
                                    in_=y_in[row0:row0 + P,
                                             tc_i * 1024:(tc_i + 2) * 1024])
                                ychunks.append(yt)
                        half_off = (tc_i % 2) * 1024
                        for tt in range(8):       # 128-wide t-ranges
                            t_idx = tc_i * 8 + tt
                            stg = pstg.tile([P, 4 * P], f32, tag="stg")
                            for j in range(4):
                                nc.tensor.transpose(
                                    stg[:, j * P:(j + 1) * P],
                                    ychunks[j][:, half_off + tt * P:half_off + (tt + 1) * P],
                                    ident[:])
                            ytile = ytpool.tile([P, 4 * P], f32r,
                                                tag=f"yt{t_idx % NROT}")
                            if t_idx % 2 == 0:
                                nc.vector.tensor_copy(out=ytile[:], in_=stg[:])
                            else:
                                nc.scalar.copy(out=ytile[:], in_=stg[:])
                            yts[t_idx] = ytile
                        for j in range(4):
                            ps = pacc.tile([P, 1024], f32, tag="acc")
                            for half, mb in enumerate((2 * tc_i, 2 * tc_i + 1)):
                                pv = ps[:, half * 512:(half + 1) * 512]
                                if mb == 0:
                                    dlist = [(d, 5 + d) for d in (0, 1, 2, 3)]
                                else:
                                    dlist = [(d, di) for di, d in
                                             enumerate((-1, 0, 1, 2, 3))]
                                first = True
                                for d, gi in dlist:
                                    ti = 4 * mb + d
                                    lhsT = yts[ti][:, j * P:(j + 1) * P]
                                    nc.tensor.matmul(
                                        pv, lhsT, gmat(gi),
                                        start=first, stop=(d == dlist[-1][0]))
                                    first = False
                            ro = rowouts[j]
                            if tc_i == 0:
                                nc.vector.scalar_tensor_tensor(
                                    out=ro[:, 0:509], in0=ps[:, 0:509], scalar=1.0,
                                    in1=bias0[:, 0:509], op0=alu.mult, op1=alu.add)
                                nc.scalar.activation(
                                    out=ro[:, 509:1021], in_=ps[:, 512:1024],
                                    func=mybir.ActivationFunctionType.Copy,
                                    bias=bias_const, scale=1.0)
                            else:
                                c0 = 1024 * tc_i - 3
                                if tc_i % 2 == 0:
                                    nc.vector.tensor_scalar(
                                        out=ro[:, c0:c0 + 1024], in0=ps[:],
                                        scalar1=1.0, scalar2=bias_const,
                                        op0=alu.mult, op1=alu.add)
                                else:
                                    nc.scalar.activation(
                                        out=ro[:, c0:c0 + 1024], in_=ps[:],
                                        func=mybir.ActivationFunctionType.Copy,
                                        bias=bias_const, scale=1.0)
                        if tc_i == 3:
                            for j in range(4):
                                row0 = (grp * 4 + j) * P
                                ro = rowouts[j]
                                nc.vector.memset(ro[:, S:T_OUT], 0.0)
                                nc.gpsimd.dma_start(
                                    out=out[row0:row0 + P, :], in_=ro[:])

            if reps == 1:
                body()
            else:
                with tc.For_i(0, reps, 1) as _r:
                    body()
            if internal:
                nc.sync.dma_start(out=yio[:], in_=bias0[0:1, 0:4])
    nc.finalize()
    return nc


def kernel(y, phi, theta, mu):
    from concourse.bass_utils import run_bass_kernel_spmd

    y = np.ascontiguousarray(np.asarray(y, np.float32))
    assert y.shape == (B_FULL, T), y.shape
    gmats, bias0, bias_const = host_constants(phi, theta, mu)
    g2 = np.ascontiguousarray(gmats.reshape(9 * P, 512))

    nc = build_program(bias_const)
    in_maps = [
        {"y": y[c * B_SH:(c + 1) * B_SH], "gmats": g2, "bias0": bias0}
        for c in range(N_CORES)
    ]
    res = run_bass_kernel_spmd(nc, in_maps, list(range(N_CORES)))
    return np.concatenate([res.results[c]["out"] for c in range(N_CORES)], axis=0)

